# revision 10
# baseline (speedup 1.0000x reference)
"""Multi-head attention Trainium2 kernel (Bass/Tile, SPMD over 8 NeuronCores).

Problem: nn_MultiHeadAttention  (B=4, S=2048, D_IN=1024, H=16, D_HEAD=64, fp32)

Sharding: core c = (batch b = c//2, head-half g = c%2).  Each core computes the
attention of 8 heads for one batch element plus the *partial* output projection
for its 512 concat-dims; the host sums the two partials per batch and adds bo.

Per-core device program (layouts chosen so no on-chip transposes are needed):
  - v computed first, in natural layout [S, h*64+d], with a ones-column
    appended per head (65th col) -> the attn@v matmul simultaneously produces
    the softmax denominator (row 64 of the [65, Sq] psum accumulator).
  - qT/kT computed in transposed layout [d(128=2 heads), S]:
        lhsT = Wq chunk (natural), rhs = X^T streamed
  - scores^T tiles [S_k=128, S_q] = k-chunk @ qT  (K = d = 64 contraction on
    partitions; head pairs live at base_partition 0/64)
  - exp on ACT: psum -> sbuf, scale = 1/sqrt(64), no max subtraction
    (|scores/8| <~ 10, fp32-safe; softmax is shift-invariant)
  - attn@v: lhsT = v'[S_k-chunk, 65], rhs = expT, accumulated over S_k tiles
  - normalize: recip = 1/denom (DVE), partition-broadcast via a K=1
    outer-product matmul, multiply, + bv -> oT [o=(h,d), S]
  - out partial = oT^T @ Wo-half: lhsT = oT chunk, rhs = Wo rows of this half

Self-loading matmuls (fp32/fp32r) support only ONE sync wait in the LW
descriptor, so every matmul operand is produced by DVE (DMA -> fp32 staging ->
DVE copy -> compute tile): all PE waits collapse onto the single DVE (or ACT)
semaphore.  A tiny "WAR-carrier" matmul after each head's normalize re-syncs
PE's DVE clock so the next accumulation start needs only its ACT wait.
"""

import os
import sys

os.environ.setdefault("MYCRO_LOCAL_CACHE", "1")
if "/opt/trn_rl_repo" not in sys.path:
    sys.path.insert(0, "/opt/trn_rl_repo")

import numpy as np

import concourse.bass as bass
from concourse import bacc
import concourse.mybir as mybir
import concourse.tile as tile
from concourse.bass_utils import run_bass_kernel_spmd

F32 = mybir.dt.float32
F32R = mybir.dt.float32r

B, S, DI, H, DH = 4, 2048, 1024, 16, 64
HC = H // 2          # heads per core = 8
HD = HC * DH         # per-core concat width = 512
P = 128
KO = DI // P         # 8 contraction k-tiles for the projections
MO = HD // P         # 4 head-pair tiles
OG = HD // P         # 4 o-dim tiles for the out projection
SCALE = 1.0 / np.sqrt(DH)

_CACHE: dict = {}


def build_program(f32r: bool, s: int = S):
    """Build the per-core Bass program. `s` is the sequence length (small
    values used for simulator-level testing)."""
    key = (f32r, s)
    if key in _CACHE:
        return _CACHE[key]

    sm = s // P                # S-tiles of 128
    chunk = min(512, s)
    sc = s // chunk            # S-chunks of 512
    ew = min(1024, s)          # exp width
    ec = s // ew               # exp chunks per row block
    cpe = ew // chunk          # 512-chunks per exp chunk

    td = F32R if f32r else F32  # dtype of every matmul-input tensor
    nc = bacc.Bacc("TRN2", target_bir_lowering=False, debug=False)

    xqT = nc.dram_tensor("xqT", [DI, s], F32, kind="ExternalInput").ap()
    xkT = nc.dram_tensor("xkT", [DI, s], F32, kind="ExternalInput").ap()
    xvT = nc.dram_tensor("xvT", [DI, s], F32, kind="ExternalInput").ap()
    wq = nc.dram_tensor("wq", [DI, HD], F32, kind="ExternalInput").ap()
    wk = nc.dram_tensor("wk", [DI, HD], F32, kind="ExternalInput").ap()
    wv = nc.dram_tensor("wv", [DI, HD], F32, kind="ExternalInput").ap()
    wo = nc.dram_tensor("wo", [HD, DI], F32, kind="ExternalInput").ap()
    bqd = nc.dram_tensor("bq", [MO, P], F32, kind="ExternalInput").ap()
    bkd = nc.dram_tensor("bk", [MO, P], F32, kind="ExternalInput").ap()
    bvd = nc.dram_tensor("bv", [MO, P], F32, kind="ExternalInput").ap()
    out = nc.dram_tensor("out", [s, DI], F32, kind="ExternalOutput").ap()

    with tile.TileContext(nc) as tc:
        with tc.tile_pool(name="persist", bufs=1) as pp:
            qT = pp.tile([P, MO, s], td, tag="qT")
            kT = pp.tile([P, MO, s], td, tag="kT")
            vsb = pp.tile([P, sm, HC, DH + 1], td, tag="v")
            oT = pp.tile([P, MO, s], td, tag="oT")
            bq_sb = pp.tile([P, MO], F32, tag="bq")
            bk_sb = pp.tile([P, MO], F32, tag="bk")
            bv_sb = pp.tile([P, MO], F32, tag="bv")
            ones = pp.tile([1, DH], F32, tag="ones")

            onesc = pp.tile([P, HC], F32, tag="onesc")
            nc.vector.memset(ones[:], 1.0)
            nc.vector.memset(onesc[:], 1.0)
            # ones columns of v' — DVE copies so PE deps stay on the DVE sem
            # (strided memset into an f32r tile fails the walrus ISA check)
            for t in range(sm):
                nc.vector.tensor_copy(vsb[:, t, :, DH : DH + 1], onesc[:, :, None])

            # biases through the DVE funnel as well
            with tc.tile_pool(name="bstage", bufs=3) as bsp:
                for bd, bt in ((bqd, bq_sb), (bkd, bk_sb), (bvd, bv_sb)):
                    bs = bsp.tile([P, MO], F32, tag="bstg", name=f"bs_{bd.name}")
                    nc.sync.dma_start(bs[:], bd.rearrange("m p -> p m"))
                    nc.vector.tensor_copy(bt[:], bs[:])

            # ---------------- Phase 1: projections ----------------
            with (
                tc.tile_pool(name="wts", bufs=1) as wp,
                tc.tile_pool(name="stg", bufs=6) as sp,
                tc.tile_pool(name="xs", bufs=4) as xp,
                tc.tile_pool(name="pj", bufs=8, space="PSUM") as pjp,
            ):
                wq_sb = [wp.tile([P, HD], td, tag=f"wq{k}", name=f"wq_sb{k}") for k in range(KO)]
                wk_sb = [wp.tile([P, HD], td, tag=f"wk{k}", name=f"wk_sb{k}") for k in range(KO)]
                wv_sb = [wp.tile([P, HD], td, tag=f"wv{k}", name=f"wv_sb{k}") for k in range(KO)]
                for ko in range(KO):
                    for wd, wt in ((wq, wq_sb), (wk, wk_sb), (wv, wv_sb)):
                        ws = sp.tile([P, HD], F32, tag="stg", name=f"ws_{wd.name}_{ko}")
                        nc.sync.dma_start(ws[:], wd[ko * P : (ko + 1) * P, :])
                        nc.vector.tensor_copy(wt[ko][:], ws[:])

                # v first (its DVE writes must precede qT/kT writes so that
                # phase 2's scores-waits cover them)
                for jtg in range(sm // 4):
                    psums = [
                        pjp.tile([P, HD], F32, tag="pp512", name=f"pjv_{jtg}_{i}")
                        for i in range(4)
                    ]
                    for ko in range(KO):
                        xs = sp.tile([P, 512], F32, tag="stg", name=f"xvs_{jtg}_{ko}")
                        nc.sync.dma_start(
                            xs[:],
                            xvT[ko * P : (ko + 1) * P, jtg * 512 : (jtg + 1) * 512],
                        )
                        xt = xp.tile([P, 512], td, tag="xv")
                        nc.vector.tensor_copy(xt[:], xs[:])
                        for j4 in range(4):
                            nc.tensor.matmul(
                                psums[j4][:],
                                xt[:, j4 * P : (j4 + 1) * P],
                                wv_sb[ko][:],
                                start=(ko == 0),
                                stop=(ko == KO - 1),
                            )
                    for j4 in range(4):
                        jt = jtg * 4 + j4
                        nc.vector.tensor_copy(
                            vsb[:, jt, :, 0:DH],
                            psums[j4].rearrange("p (h d) -> p h d", h=HC),
                        )

                # q and k in transposed layout: psum [128 (pair d), 512 (S)]
                for name, xT, w_sb, dst, b_sb in (
                    ("q", xqT, wq_sb, qT, bq_sb),
                    ("k", xkT, wk_sb, kT, bk_sb),
                ):
                    for nch in range(sc):
                        psums = [
                            pjp.tile([P, chunk], F32, tag="pp512", name=f"pj_{name}_{nch}_{i}")
                            for i in range(MO)
                        ]
                        for ko in range(KO):
                            xs = sp.tile([P, chunk], F32, tag="stg", name=f"x{name}s_{nch}_{ko}")
                            nc.sync.dma_start(
                                xs[:],
                                xT[ko * P : (ko + 1) * P, nch * chunk : (nch + 1) * chunk],
                            )
                            xt = xp.tile([P, chunk], td, tag="xqk")
                            nc.vector.tensor_copy(xt[:], xs[:])
                            for mo in range(MO):
                                nc.tensor.matmul(
                                    psums[mo][:],
                                    w_sb[ko][:, mo * P : (mo + 1) * P],
                                    xt[:],
                                    start=(ko == 0),
                                    stop=(ko == KO - 1),
                                )
                        for mo in range(MO):
                            nc.vector.tensor_scalar_add(
                                dst[:, mo, nch * chunk : (nch + 1) * chunk],
                                psums[mo][:],
                                b_sb[:, mo : mo + 1],
                            )

            # ---------------- Phase 2: attention ----------------
            # Sequential heads; scores psum triple-buffered for ACT overlap.
            # The partition-broadcast psum tiles share the scores pool slots
            # (same tag) so the total stays within 8 PSUM banks.
            with (
                tc.tile_pool(name="et", bufs=4) as ep,
                tc.tile_pool(name="rc", bufs=4) as rp,
                tc.tile_pool(name="scp", bufs=3, space="PSUM") as scp,
                tc.tile_pool(name="avp", bufs=1, space="PSUM") as avp,
            ):
                for h in range(HC):
                    pr, r = h // 2, h % 2
                    lo, hi = r * DH, (r + 1) * DH
                    for c2 in range(ec):
                        av = avp.tile([P, ew], F32, tag="av", name=f"av_{h}_{c2}")
                        for t in range(sm):
                            sc_ps = scp.tile([P, ew], F32, tag="sc", name=f"sc_{h}_{c2}_{t}")
                            for cc in range(cpe):
                                q0 = c2 * ew + cc * chunk
                                nc.tensor.matmul(
                                    sc_ps[:, cc * chunk : (cc + 1) * chunk],
                                    kT[lo:hi, pr, t * P : (t + 1) * P],
                                    qT[lo:hi, pr, q0 : q0 + chunk],
                                    start=True,
                                    stop=True,
                                )
                            et = ep.tile([P, ew], td, tag="et", name=f"et_{h}_{c2}_{t}")
                            nc.scalar.activation(
                                et[:],
                                sc_ps[:],
                                mybir.ActivationFunctionType.Exp,
                                scale=float(SCALE),
                            )
                            for cc in range(cpe):
                                nc.tensor.matmul(
                                    av[0 : DH + 1, cc * chunk : (cc + 1) * chunk],
                                    vsb[:, t, h, :],
                                    et[:, cc * chunk : (cc + 1) * chunk],
                                    start=(t == 0),
                                    stop=(t == sm - 1),
                                )
                        # normalize + bias -> oT
                        for cc in range(cpe):
                            q0 = c2 * ew + cc * chunk
                            rec = rp.tile([1, chunk], F32, tag="rec", name=f"rec_{h}_{c2}_{cc}")
                            nc.vector.reciprocal(
                                rec[:], av[DH : DH + 1, cc * chunk : (cc + 1) * chunk]
                            )
                            bc = scp.tile([DH, chunk], F32, tag="sc", name=f"bc_{h}_{c2}_{cc}")
                            nc.tensor.matmul(
                                bc[:], ones[0:1, 0:DH], rec[0:1, :],
                                start=True, stop=True,
                            )
                            bcs = rp.tile([DH, chunk], F32, tag="bcs", name=f"bcs_{h}_{c2}_{cc}")
                            nc.vector.tensor_copy(bcs[:], bc[:])
                            nc.vector.tensor_tensor(
                                oT[lo:hi, pr, q0 : q0 + chunk],
                                av[0:DH, cc * chunk : (cc + 1) * chunk],
                                bcs[:],
                                mybir.AluOpType.mult,
                            )
                            nc.vector.tensor_scalar_add(
                                oT[lo:hi, pr, q0 : q0 + chunk],
                                oT[lo:hi, pr, q0 : q0 + chunk],
                                bv_sb[lo:hi, pr : pr + 1],
                            )
                        # WAR-carrier: a tiny matmul whose only deps are on the
                        # DVE sem re-syncs PE past this block's psum reads, so
                        # the next (h,c2) start=True matmul needs only its ACT
                        # wait (self-loading matmuls allow a single LW wait).
                        nc.tensor.matmul(
                            av[0:1, 0:DH],
                            ones[0:1, 0:1],
                            ones[0:1, 0:DH],
                            start=True,
                            stop=True,
                        )

            # ---------------- Phase 3: output projection ----------------
            with (
                tc.tile_pool(name="wos", bufs=2) as wsp,
                tc.tile_pool(name="wo", bufs=1) as wop,
                tc.tile_pool(name="os", bufs=4) as osp,
                tc.tile_pool(name="po", bufs=4, space="PSUM") as pop,
            ):
                wo_sb = [wop.tile([P, DI], td, tag=f"wo{og}", name=f"wo_sb{og}") for og in range(OG)]
                for og in range(OG):
                    ws = wsp.tile([P, DI], F32, tag="wostg", name=f"wos_{og}")
                    nc.sync.dma_start(ws[:], wo[og * P : (og + 1) * P, :])
                    nc.vector.tensor_copy(wo_sb[og][:], ws[:])
                for m in range(sm):
                    for n2 in range(DI // 512):
                        ps = pop.tile([P, 512], F32, tag="po", name=f"po_{m}_{n2}")
                        for og in range(OG):
                            nc.tensor.matmul(
                                ps[:],
                                oT[:, og, m * P : (m + 1) * P],
                                wo_sb[og][:, n2 * 512 : (n2 + 1) * 512],
                                start=(og == 0),
                                stop=(og == OG - 1),
                            )
                        ot = osp.tile([P, 512], F32, tag="os", name=f"os_{m}_{n2}")
                        nc.vector.tensor_copy(ot[:], ps[:])
                        nc.sync.dma_start(
                            out[m * P : (m + 1) * P, n2 * 512 : (n2 + 1) * 512],
                            ot[:],
                        )

    nc.compile()
    _CACHE[key] = nc
    return nc


def make_in_maps(query, key, value, Wq, bq, Wk, bk, Wv, bv, Wo):
    """Shard the full inputs into 8 per-core input maps."""
    f = lambda a: np.ascontiguousarray(np.asarray(a, dtype=np.float32))
    in_maps = []
    for c in range(8):
        b, g = c // 2, c % 2
        hs = slice(g * HC, (g + 1) * HC)
        in_maps.append(
            {
                "xqT": f(np.asarray(query[b]).T),
                "xkT": f(np.asarray(key[b]).T),
                "xvT": f(np.asarray(value[b]).T),
                "wq": f(np.transpose(np.asarray(Wq)[hs], (1, 0, 2)).reshape(DI, HD)),
                "wk": f(np.transpose(np.asarray(Wk)[hs], (1, 0, 2)).reshape(DI, HD)),
                "wv": f(np.transpose(np.asarray(Wv)[hs], (1, 0, 2)).reshape(DI, HD)),
                "wo": f(np.asarray(Wo)[g * HD : (g + 1) * HD]),
                "bq": f(np.asarray(bq)[hs].reshape(MO, P)),
                "bk": f(np.asarray(bk)[hs].reshape(MO, P)),
                "bv": f(np.asarray(bv)[hs].reshape(MO, P)),
            }
        )
    return in_maps


# Default numeric mode for the graded path. float32r runs the PE at 4x the
# fp32 rate; enabled after verifying accuracy against the oracle on HW.
F32R_MODE = True


def run(inputs: dict, f32r: bool | None = None, trace: bool = False):
    """Run on hardware; returns (full_output, BassKernelResults)."""
    if f32r is None:
        f32r = F32R_MODE
    nc = build_program(f32r)
    in_maps = make_in_maps(
        inputs["query"], inputs["key"], inputs["value"],
        inputs["Wq"], inputs["bq"], inputs["Wk"], inputs["bk"],
        inputs["Wv"], inputs["bv"], inputs["Wo"],
    )
    res = run_bass_kernel_spmd(nc, in_maps, core_ids=list(range(8)), trace=trace)
    bo = np.asarray(inputs["bo"], dtype=np.float32)
    full = np.empty((B, S, DI), dtype=np.float32)
    for b in range(B):
        full[b] = res.results[2 * b]["out"] + res.results[2 * b + 1]["out"] + bo
    return full, res


def kernel(**inputs) -> np.ndarray:
    full, _ = run(inputs)
    return full
